# revision 2
# baseline (speedup 1.0000x reference)
"""ContextWeaver: context[i, j] = relu(sum_{k,d} node[i,k,d] * edge[j,k,d]), diag zeroed.

Strategy (8 NeuronCores, SPMD):
  - Shard node rows 8-way (1024 rows/core); replicate edge^T per core with a
    per-core column rotation of c*1024 -- the instruction stream is identical
    on all cores. relu, dequant, and diagonal zeroing happen on the HOST.
  - Precision plan (gate is rel_err < 2e-2 vs the fp32 reference, normalized
    by the GLOBAL output max ~45.8):
      inputs  -> fp16 on host; the 127/64 quantization scale is folded into
                 the node operand, so PSUM holds 127/64 * score and the
                 drains are bare fp32->int8 copies,
      scores  -> fp32 in PSUM,
      output  -> int8 = round(score * 127/64): max abs err 0.25 score units
                 = 0.55% of the global max. |score| > 64 is a >8-sigma event.
  - Contraction dim is 64 (= K*D); two independent 64-row matmuls packed in
    the 128x128 PE via tile_position row tiling: partitions 0-63 compute
    local columns [0, 4096), partitions 64-127 compute [4096, 8192).
  - PSUM-read bandwidth is the hard floor on TRN2 (only DVE/ACT can read
    PSUM, 1 fp32/cycle/partition each; DVE @0.96GHz, ACT @1.2GHz). Balanced
    split: DVE 30 and ACT 34 of the 64 [128,1024] drain chunks per core
    (back-to-back cadences 1131ns vs 995ns -> 33.9us each). Engine choice
    per chunk follows a fixed global pattern, decoupled from which PE
    row-group produced the chunk.
  - Strip 0 chunk 0 and strip 7 chunk 3 are split into 512-wide drains on
    BOTH engines: opens the output-DMA window earlier at the start, and
    closes the drain streams simultaneously at the end (the final DMA gates
    the fixed ~8.5us postamble).
  - ALL dma_start issues on SP except the node input (ACT ring) so the two
    input streams run in parallel. Output pieces sized [128, 4096] midstream
    (2KB/partition-row lines at 4096 int8 -- efficient), finer on strips 0/7.
  - Host: rotate each slab back, dequant * 64/127, relu, zero diagonal.
"""

import os as _os

_os.environ.setdefault("JAX_PLATFORMS", "axon,cpu")

import numpy as np

import concourse.bass as bass
import concourse.mybir as mybir
import concourse.tile as tile
from concourse import bacc
from concourse.bass_utils import run_bass_kernel_spmd

N = 8192          # nodes
F = 64            # contraction (K*D = 2*32)
NCORES = 8
SHARD = N // NCORES        # 1024 rows per core
HALF = N // 2              # 4096 local columns per PE row-group
MT = 128                   # output-row strip height
NT = 512                   # matmul moving free dim (one PSUM bank fp32)
NT2 = 2 * NT               # 1024-col drain/psum-tile granularity

QSCALE = 64.0              # int8 full-scale in score units
QMUL = 127.0 / QSCALE      # device-side multiplier before int8 cast

F32 = mybir.dt.float32
FP16 = mybir.dt.float16
I8 = mybir.dt.int8


def build_nc():
    nc = bacc.Bacc("TRN2", target_bir_lowering=False, debug=False)

    node2_d = nc.dram_tensor("node2", [128, SHARD], FP16, kind="ExternalInput")
    edge2_d = nc.dram_tensor("edge2", [128, HALF], FP16, kind="ExternalInput")
    out_d = nc.dram_tensor("out", [SHARD, N], I8, kind="ExternalOutput")

    n_strips = SHARD // MT           # 8
    n_chunks = HALF // NT2           # 4 chunk-pairs per strip

    # Drain-engine pattern: per strip, 8 drains in issue order
    # [a0, b0, a1, b1, a2, b2, a3, b3] (a = lo cols, b = hi cols).
    # 'D' = DVE tensor_copy, 'A' = ACT activation-Copy.
    # Six strips at 4D/4A, strips 3 and 7 at 3D/5A -> 30 DVE / 34 ACT
    # (1131ns vs 995ns back-to-back cadence -> both engines ~33.9us busy).
    PAT_EVEN = ['D', 'A', 'D', 'A', 'D', 'A', 'D', 'A']
    PAT_HEAVY = ['D', 'A', 'A', 'D', 'A', 'D', 'A', 'A']

    def drain(eng, dst_ap, src_ap):
        if eng == 'D':
            nc.vector.tensor_copy(dst_ap, src_ap)
        else:
            nc.scalar.activation(
                dst_ap, src_ap, mybir.ActivationFunctionType.Copy, 0.0, 1.0,
            )

    with tile.TileContext(nc) as tc:
        with (
            tc.tile_pool(name="consts", bufs=1) as consts,
            tc.tile_pool(name="outp", bufs=4) as outp,
            tc.tile_pool(name="psp", bufs=2, space=bass.MemorySpace.PSUM) as psp,
        ):
            node_sb = consts.tile([128, SHARD], FP16)
            edge_sb = consts.tile([128, HALF], FP16)

            # node on the ACT ring, edge on the SP ring: the two input
            # streams run in parallel. node rows arrive pre-duplicated from
            # the host ([128, SHARD]), split by partition half so the lo
            # matmuls are not gated on the hi half's completion semaphore.
            # The first edge piece is small so its completion semaphore
            # (~2.3us receipt latency) fires as early as possible; it feeds
            # the first chunk of BOTH row-groups.
            nc.scalar.dma_start(out=node_sb[0:64, :], in_=node2_d[0:64, :])
            nc.scalar.dma_start(out=node_sb[64:128, :], in_=node2_d[64:128, :])
            nc.sync.dma_start(out=edge_sb[:, 0:NT], in_=edge2_d[:, 0:NT])
            nc.sync.dma_start(out=edge_sb[:, NT:NT2], in_=edge2_d[:, NT:NT2])
            nc.sync.dma_start(out=edge_sb[:, NT2:2 * NT2],
                              in_=edge2_d[:, NT2:2 * NT2])
            nc.sync.dma_start(out=edge_sb[:, 2 * NT2:],
                              in_=edge2_d[:, 2 * NT2:])

            for m in range(n_strips):
                strip = outp.tile([128, N], I8)
                lhs_lo = node_sb[0:64, m * MT:(m + 1) * MT]
                lhs_hi = node_sb[64:128, m * MT:(m + 1) * MT]
                pat = PAT_HEAVY if m in (3, 7) else PAT_EVEN
                first = (m == 0)
                last = (m == n_strips - 1)
                for n in range(n_chunks):
                    ps_a = psp.tile([128, NT2], F32)
                    ps_b = psp.tile([128, NT2], F32)
                    c0, c1 = n * NT2, n * NT2 + NT
                    nc.tensor.matmul(
                        ps_a[:, 0:NT], lhs_lo, edge_sb[0:64, c0:c0 + NT],
                        start=True, stop=True, tile_position=(0, 0),
                    )
                    nc.tensor.matmul(
                        ps_b[:, 0:NT], lhs_hi, edge_sb[64:128, c0:c0 + NT],
                        start=True, stop=True, tile_position=(64, 0),
                    )
                    nc.tensor.matmul(
                        ps_a[:, NT:NT2], lhs_lo, edge_sb[0:64, c1:c1 + NT],
                        start=True, stop=True, tile_position=(0, 0),
                    )
                    nc.tensor.matmul(
                        ps_b[:, NT:NT2], lhs_hi, edge_sb[64:128, c1:c1 + NT],
                        start=True, stop=True, tile_position=(64, 0),
                    )
                    ea, eb = pat[2 * n], pat[2 * n + 1]
                    if (first or last) and n == (0 if first else n_chunks - 1):
                        # split into 512-wide drains on both engines:
                        # strip 0 -> both engines start ASAP (and the first
                        # output DMA unblocks one matmul earlier); strip 7
                        # -> both drain streams close simultaneously so the
                        # postamble-gating final DMAs start earlier.
                        drain('D', strip[:, c0:c0 + NT], ps_a[:, 0:NT])
                        drain('A', strip[:, HALF + c0:HALF + c0 + NT],
                              ps_b[:, 0:NT])
                        drain('D', strip[:, c1:c1 + NT], ps_a[:, NT:NT2])
                        drain('A', strip[:, HALF + c1:HALF + c1 + NT],
                              ps_b[:, NT:NT2])
                    else:
                        drain(ea, strip[:, c0:c0 + NT2], ps_a[:])
                        drain(eb, strip[:, HALF + c0:HALF + c0 + NT2],
                              ps_b[:])
                # output pieces on SP in readiness order; strip 0 finer so
                # the DMA window opens early, strip 7 finer so the final
                # transfer lands right after the last drains
                if first:
                    pieces = [(0, 512), (4096, 4608), (512, 1024),
                              (4608, 5120), (1024, 2048), (5120, 6144),
                              (2048, 4096), (6144, 8192)]
                elif last:
                    pieces = [(0, 2048), (4096, 6144), (2048, 3072),
                              (6144, 7168), (3072, 3584), (7168, 7680),
                              (3584, 4096), (7680, 8192)]
                else:
                    pieces = [(0, HALF), (HALF, N)]
                for lo, hi in pieces:
                    nc.sync.dma_start(
                        out=out_d[m * MT:(m + 1) * MT, lo:hi],
                        in_=strip[:, lo:hi],
                    )

    nc.compile()
    return nc


_NC = None


def _get_nc():
    global _NC
    if _NC is None:
        _NC = build_nc()
    return _NC


def make_in_maps(node_features: np.ndarray, edge_features: np.ndarray):
    node = np.ascontiguousarray(node_features, dtype=np.float32).reshape(N, F)
    edge = np.ascontiguousarray(edge_features, dtype=np.float32).reshape(N, F)
    edge_t = np.ascontiguousarray(edge.T).astype(np.float16)    # [64, 8192]

    in_maps = []
    for c in range(NCORES):
        # quantization scale folded into the node operand (scale-invariant
        # under fp16 relative rounding): PSUM then holds 127/64 * score.
        # Rows pre-duplicated for the two PE row-groups.
        node_t = (node[c * SHARD:(c + 1) * SHARD].T * QMUL).astype(np.float16)
        node2 = np.ascontiguousarray(
            np.concatenate([node_t, node_t], axis=0)            # [128, 1024]
        )
        et = np.roll(edge_t, -c * SHARD, axis=1)   # local col j' = global (j'+c*1024)%N
        edge2 = np.ascontiguousarray(
            np.concatenate([et[:, :HALF], et[:, HALF:]], axis=0)
        )
        in_maps.append({"node2": node2, "edge2": edge2})
    return in_maps


def kernel(node_features: np.ndarray, edge_features: np.ndarray) -> np.ndarray:
    nc = _get_nc()
    in_maps = make_in_maps(node_features, edge_features)
    res = run_bass_kernel_spmd(nc, in_maps, core_ids=list(range(NCORES)))
    out = np.empty((N, N), np.float32)
    dq = np.float32(QSCALE / 127.0)
    for c in range(NCORES):
        slab = np.roll(res.results[c]["out"], c * SHARD, axis=1)
        slab = slab.astype(np.float32) * dq
        np.maximum(slab, 0.0, out=slab)
        out[c * SHARD:(c + 1) * SHARD] = slab
    np.fill_diagonal(out, 0.0)
    return out
